# revision 9
# baseline (speedup 1.0000x reference)
"""Trainium2 Bass kernel for masked Luong-'general' attention.

reference math (per batch b):
    scores = softmax(mask(h_t @ W_in @ h_s^T))       # [T, S]
    context = scores @ h_s                           # [T, D]
returns (context, scores).

Sharding: 8 cores = 4 batches x 2 halves of the target sequence.
Each core computes a [2048, 4096] score block + [2048, 256] context block.

Per-core dataflow (all matmuls in float32r = fp22, full PE rate):
  G^T[d,q]   = W_in(native lhsT) . h_tT          (h_tT via PE transposes)
  S_T[s,q]   = h_sT-chunks(lhsT) . G^T           (PSUM)
  P_T[s,q]   = exp(S_T + maskbias[s])            (one ACT op: mask+exp+drain)
  ctx|Z[q,.] = P_T-chunks(lhsT) . [h_s | 1]      (ones col gives softmax denom)
  scores     = transpose(P_T) * (1/Z)            (PE transpose + DVE drain)
"""

import os
import numpy as np

import concourse.bass as bass
import concourse.tile as tile
import concourse.mybir as mybir
from concourse.bass_utils import run_bass_kernel_spmd

F32 = mybir.dt.float32
F32R = mybir.dt.float32r
EXP = mybir.ActivationFunctionType.Exp

N_CORES = 8
B, TGT, SRC, D = 4, 4096, 4096, 256
T_SH = TGT * B // N_CORES  # 2048 target rows per core
NEG_BIG = 1.0e11


def _r(ap):
    return ap.bitcast(F32R)


def _split_multi_waits(nc, max_waits=1):
    """This walrus build's CTRL templates (Drain etc.) only accept one sync
    wait; peel extra waits onto preceding NoOps on the same engine."""
    for f in nc.m.functions:
        for bb in f.blocks:
            insts = bb.instructions
            i = 0
            while i < len(insts):
                inst = insts[i]
                si = inst.sync_info
                if si is not None and len(si.on_wait) > max_waits:
                    waits = list(si.on_wait)
                    keep = waits[-max_waits:]
                    extra = waits[:-max_waits]
                    nops = []
                    for j in range(0, len(extra), max_waits):
                        nops.append(
                            mybir.InstNoOp(
                                name=f"{inst.name}_ws{j}",
                                engine=inst.engine,
                                bass_nofuse=True,
                                sync_info=mybir.SyncInfo(
                                    on_wait=extra[j : j + max_waits], on_update=[]
                                ),
                            )
                        )
                    inst.sync_info = mybir.SyncInfo(
                        on_wait=keep, on_update=list(si.on_update)
                    )
                    for k, nop in enumerate(nops):
                        insts.insert(i + k, nop)
                    i += len(nops)
                i += 1


def build_nc(t_sh=T_SH, s_len=SRC, q_blk=512, split_waits=True):
    """Build the per-core SPMD program. All cores run the same NEFF."""
    d = D
    nqt = t_sh // 128  # q tiles (of 128)
    nst = s_len // 128  # s tiles
    nqb = t_sh // q_blk  # q blocks held in SBUF at once
    qsub = q_blk // 128  # q subtiles per block

    nc = bass.Bass(trn_type="TRN2", target_bir_lowering=False, debug=False)

    ht_d = nc.dram_tensor("h_t", [t_sh, d], F32R, kind="ExternalInput")
    hsa_d = nc.dram_tensor("h_s_aug", [s_len, d + 2], F32R, kind="ExternalInput")
    mb_d = nc.dram_tensor("maskbias", [s_len], F32, kind="ExternalInput")
    w_d = nc.dram_tensor("W_in", [d, d], F32R, kind="ExternalInput")
    id_d = nc.dram_tensor("ident", [128, 128], F32R, kind="ExternalInput")
    sc_d = nc.dram_tensor("scores", [t_sh, s_len], F32, kind="ExternalOutput")
    cx_d = nc.dram_tensor("context", [t_sh, d], F32, kind="ExternalOutput")

    with tile.TileContext(nc) as tc:
        with (
            tc.tile_pool(name="persist", bufs=1) as pp,
            tc.tile_pool(name="pblk", bufs=1) as bp,
            tc.tile_pool(name="pout", bufs=1) as op,
            tc.tile_pool(name="work", bufs=2) as wp,
            tc.tile_pool(name="spsum", bufs=2, space="PSUM") as sp,
            tc.tile_pool(name="tpsum", bufs=2, space="PSUM") as tpp,
            tc.tile_pool(name="vpsum", bufs=2, space="PSUM") as vp,
        ):
            # ---- persistent loads ----
            w_sb = pp.tile([128, 2, d], F32R, tag="w")
            nc.sync.dma_start(w_sb[:], w_d.ap().rearrange("(c p) f -> p c f", p=128))
            id_sb = pp.tile([128, 128], F32R, tag="ident")
            nc.sync.dma_start(id_sb[:], id_d.ap())
            mb_sb = pp.tile([128, nst], F32, tag="mb")
            nc.sync.dma_start(mb_sb[:], mb_d.ap().rearrange("(c p) -> p c", p=128))
            hsa_sb = pp.tile([128, nst, d + 2], F32R, tag="hsa")
            nc.sync.dma_start(
                hsa_sb[:], hsa_d.ap().rearrange("(c p) f -> p c f", p=128)
            )

            # ---- transposes of h_t and h_s (via PE) ----
            hsT_sb = pp.tile([128, 2, s_len], F32R, tag="hsT")
            gt_sb = pp.tile([128, 2, t_sh], F32R, tag="gt")

            with tc.tile_pool(name="setup", bufs=1) as stp:
                ht_sb = stp.tile([128, nqt, d], F32R, tag="ht")
                nc.sync.dma_start(
                    ht_sb[:], ht_d.ap().rearrange("(c p) f -> p c f", p=128)
                )
                htT_sb = stp.tile([128, 2, t_sh], F32R, tag="htT")
                for tch in range(2):
                    for g in range(0, nqt, 4):
                        n = min(4, nqt - g)
                        ps = tpp.tile([128, 512], F32, tag="tps")
                        for k in range(n):
                            c = g + k
                            nc.tensor.transpose(
                                _r(ps[:, k * 128 : (k + 1) * 128]),
                                ht_sb[:, c, tch * 128 : (tch + 1) * 128],
                                id_sb[:],
                            )
                        nc.vector.tensor_copy(
                            htT_sb[:, tch, g * 128 : (g + n) * 128], ps[:, : n * 128]
                        )
                for dch in range(2):
                    for g in range(0, nst, 4):
                        n = min(4, nst - g)
                        ps = tpp.tile([128, 512], F32, tag="tps")
                        for k in range(n):
                            c = g + k
                            nc.tensor.transpose(
                                _r(ps[:, k * 128 : (k + 1) * 128]),
                                hsa_sb[:, c, dch * 128 : (dch + 1) * 128],
                                id_sb[:],
                            )
                        nc.vector.tensor_copy(
                            hsT_sb[:, dch, g * 128 : (g + n) * 128], ps[:, : n * 128]
                        )

                # ---- G^T = (h_t @ W_in)^T : [d, q] ----
                for dt_ in range(2):
                    for q0 in range(0, t_sh, 512):
                        qn = min(512, t_sh - q0)
                        ps = sp.tile([128, q_blk if q_blk > 512 else 512], F32, tag="sps")
                        for tch in range(2):
                            nc.tensor.matmul(
                                ps[:, :qn],
                                w_sb[:, tch, dt_ * 128 : (dt_ + 1) * 128],
                                htT_sb[:, tch, q0 : q0 + qn],
                                start=(tch == 0),
                                stop=(tch == 1),
                            )
                        nc.vector.tensor_copy(gt_sb[:, dt_, q0 : q0 + qn], ps[:, :qn])

            # ---- main loop over q blocks ----
            for qb in range(nqb):
                q0 = qb * q_blk
                p_blk = bp.tile([128, nst * q_blk], F32R, tag="pblk")
                for st in range(nst):
                    ps = sp.tile([128, q_blk if q_blk > 512 else 512], F32, tag="sps")
                    for dch in range(2):
                        nc.tensor.matmul(
                            ps[:, :q_blk],
                            hsT_sb[:, dch, st * 128 : (st + 1) * 128],
                            gt_sb[:, dch, q0 : q0 + q_blk],
                            start=(dch == 0),
                            stop=(dch == 1),
                        )
                    nc.scalar.activation(
                        p_blk[:, st * q_blk : (st + 1) * q_blk],
                        ps[:, :q_blk],
                        EXP,
                        bias=mb_sb[:, st : st + 1],
                        scale=1.0,
                    )
                for qt in range(qsub):
                    qq = q0 + qt * 128
                    pv = vp.tile([128, d + 2], F32, tag="pv")
                    for st in range(nst):
                        nc.tensor.matmul(
                            pv[:],
                            p_blk[
                                :, st * q_blk + qt * 128 : st * q_blk + (qt + 1) * 128
                            ],
                            hsa_sb[:, st, :],
                            start=(st == 0),
                            stop=(st == nst - 1),
                        )
                    invz = wp.tile([128, 1], F32, tag="invz")
                    nc.vector.reciprocal(invz[:], pv[:, d : d + 1])
                    cxt = wp.tile([128, d], F32, tag="cxt")
                    nc.vector.tensor_scalar_mul(cxt[:], pv[:, 0:d], invz[:])
                    nc.sync.dma_start(cx_d.ap()[qq : qq + 128, :], cxt[:])

                    pout = op.tile([128, s_len], F32, tag="pout")
                    for sg in range(0, nst, 4):
                        n = min(4, nst - sg)
                        tp = tpp.tile([128, 512], F32, tag="tps")
                        for k in range(n):
                            st = sg + k
                            nc.tensor.transpose(
                                _r(tp[:, k * 128 : (k + 1) * 128]),
                                p_blk[
                                    :,
                                    st * q_blk
                                    + qt * 128 : st * q_blk
                                    + (qt + 1) * 128,
                                ],
                                id_sb[:],
                            )
                        nc.vector.tensor_scalar_mul(
                            pout[:, sg * 128 : (sg + n) * 128],
                            tp[:, : n * 128],
                            invz[:],
                        )
                    nc.sync.dma_start(sc_d.ap()[qq : qq + 128, :], pout[:])

    if split_waits:
        _split_multi_waits(nc)
    return nc


_NC_CACHE = {}


def _get_nc():
    if "nc" not in _NC_CACHE:
        _NC_CACHE["nc"] = build_nc()
    return _NC_CACHE["nc"]


def _install_ntff_hook():
    """Dev-only: recreate the missing antenv.axon_hooks NTFF profile hook via
    ctypes on the injected axon .so, so trace=True yields exec_time_ns."""
    import contextlib
    import ctypes
    import sys
    import types

    try:
        import antenv.axon_hooks  # noqa: F401

        return
    except ImportError:
        pass

    so_path = "/opt/axon/libaxon_pjrt.so"
    hook = None
    try:
        lib = ctypes.CDLL(so_path)
        if hasattr(lib, "axon_start_nrt_profile"):
            lib.axon_start_nrt_profile.argtypes = [
                ctypes.POINTER(ctypes.c_int64),
                ctypes.c_size_t,
            ]
            lib.axon_start_nrt_profile.restype = ctypes.c_int64
            lib.axon_stop_nrt_profile.argtypes = [ctypes.c_char_p]
            lib.axon_stop_nrt_profile.restype = ctypes.c_int64

            @contextlib.contextmanager
            def _hook(output_dir, device_ids):
                import jax

                jax.devices()
                if device_ids:
                    ids = (ctypes.c_int64 * len(device_ids))(*device_ids)
                    rc = lib.axon_start_nrt_profile(ids, len(device_ids))
                else:
                    rc = lib.axon_start_nrt_profile(None, 0)
                if rc != 0:
                    raise RuntimeError(f"axon_start_nrt_profile rc={rc}")
                try:
                    yield
                finally:
                    n = lib.axon_stop_nrt_profile(str(output_dir).encode())
                    if n < 0:
                        raise RuntimeError(f"axon_stop_nrt_profile rc={n}")
                    print(f"profile: {n} file(s) written to {output_dir}", file=sys.stderr)

            hook = _hook
    except OSError:
        pass

    mod = types.ModuleType("antenv.axon_hooks")
    mod._hook = hook
    mod.get_axon_ntff_profile_hook = lambda: mod._hook
    mod.set_axon_ntff_profile_hook = lambda h: setattr(mod, "_hook", h)
    sys.modules["antenv.axon_hooks"] = mod
    import antenv

    antenv.axon_hooks = mod

    # artifact upload has no destination in this sandbox
    import concourse.bass_utils as bu

    bu.upload_artifacts = lambda tmpdir: "local://skipped"


def kernel(h_t, h_s, m_s, W_in):
    h_t = np.ascontiguousarray(np.asarray(h_t, dtype=np.float32))
    h_s = np.ascontiguousarray(np.asarray(h_s, dtype=np.float32))
    m_s = np.asarray(m_s)
    W_in = np.ascontiguousarray(np.asarray(W_in, dtype=np.float32))

    nc = _get_nc()
    ident = np.eye(128, dtype=np.float32)
    aug = np.zeros((SRC, 2), dtype=np.float32)
    aug[:, 0] = 1.0
    hsa = [np.ascontiguousarray(np.concatenate([h_s[b], aug], axis=1)) for b in range(B)]
    mb = [
        np.ascontiguousarray((m_s[b].astype(np.float32) - 1.0) * NEG_BIG)
        for b in range(B)
    ]

    in_maps = []
    for c in range(N_CORES):
        b, half = c // 2, c % 2
        in_maps.append(
            {
                "h_t": np.ascontiguousarray(
                    h_t[b, half * T_SH : (half + 1) * T_SH, :]
                ),
                "h_s_aug": hsa[b],
                "maskbias": mb[b],
                "W_in": W_in,
                "ident": ident,
            }
        )

    trace = os.environ.get("TRNK_TRACE", "0") == "1"
    if trace:
        _install_ntff_hook()
    res = run_bass_kernel_spmd(
        nc, in_maps, core_ids=list(range(N_CORES)), trace=trace
    )
    kernel.last_exec_time_ns = res.exec_time_ns
    kernel.last_results = res

    scores = np.empty((B, TGT, SRC), dtype=np.float32)
    context = np.empty((B, TGT, D), dtype=np.float32)
    for c in range(N_CORES):
        b, half = c // 2, c % 2
        scores[b, half * T_SH : (half + 1) * T_SH, :] = res.results[c]["scores"]
        context[b, half * T_SH : (half + 1) * T_SH, :] = res.results[c]["context"]
    return context, scores


# revision 10
# speedup vs baseline: 1.3083x; 1.3083x over previous
"""Trainium2 Bass kernel for masked Luong-'general' attention.

reference math (per batch b):
    scores = softmax(mask(h_t @ W_in @ h_s^T))       # [T, S]
    context = scores @ h_s                           # [T, D]
returns (context, scores).

Sharding: 8 cores = 4 batches x 2 halves of the target sequence.
Each core computes a [2048, 4096] score block + [2048, 256] context block.

Per-core dataflow (all matmuls in float32r = fp22, full PE rate):
  G^T[d,q]   = W_in(native lhsT) . h_tT          (h_tT via PE transposes)
  S_T[s,q]   = h_sT-chunks(lhsT) . G^T           (PSUM)
  P_T[s,q]   = exp(S_T + maskbias[s])            (one ACT op: mask+exp+drain)
  ctx|Z[q,.] = P_T-chunks(lhsT) . [h_s | 1]      (ones col gives softmax denom)
  scores     = transpose(P_T) * (1/Z)            (PE transpose + DVE drain)
"""

import os
import numpy as np

import concourse.bass as bass
import concourse.tile as tile
import concourse.mybir as mybir
from concourse.bass_utils import run_bass_kernel_spmd

F32 = mybir.dt.float32
F32R = mybir.dt.float32r
BF16 = mybir.dt.bfloat16
EXP = mybir.ActivationFunctionType.Exp

N_CORES = 8
B, TGT, SRC, D = 4, 4096, 4096, 256
T_SH = TGT * B // N_CORES  # 2048 target rows per core
NEG_BIG = 1.0e11


def _r(ap):
    return ap.bitcast(F32R)


def _split_multi_waits(nc, max_waits=1):
    """This walrus build's CTRL templates (Drain etc.) only accept one sync
    wait; peel extra waits onto preceding NoOps on the same engine."""
    for f in nc.m.functions:
        for bb in f.blocks:
            insts = bb.instructions
            i = 0
            while i < len(insts):
                inst = insts[i]
                si = inst.sync_info
                if si is not None and len(si.on_wait) > max_waits:
                    waits = list(si.on_wait)
                    keep = waits[-max_waits:]
                    extra = waits[:-max_waits]
                    nops = []
                    for j in range(0, len(extra), max_waits):
                        nops.append(
                            mybir.InstNoOp(
                                name=f"{inst.name}_ws{j}",
                                engine=inst.engine,
                                bass_nofuse=True,
                                sync_info=mybir.SyncInfo(
                                    on_wait=extra[j : j + max_waits], on_update=[]
                                ),
                            )
                        )
                    inst.sync_info = mybir.SyncInfo(
                        on_wait=keep, on_update=list(si.on_update)
                    )
                    for k, nop in enumerate(nops):
                        insts.insert(i + k, nop)
                    i += len(nops)
                i += 1


def build_nc(t_sh=T_SH, s_len=SRC, q_blk=512, split_waits=True):
    """Build the per-core SPMD program. All cores run the same NEFF."""
    d = D
    nqt = t_sh // 128  # q tiles (of 128)
    nst = s_len // 128  # s tiles
    nqb = t_sh // q_blk  # q blocks held in SBUF at once
    qsub = q_blk // 128  # q subtiles per block

    nc = bass.Bass(trn_type="TRN2", target_bir_lowering=False, debug=False)

    ht_d = nc.dram_tensor("h_t", [t_sh, d], F32R, kind="ExternalInput")
    hsa_d = nc.dram_tensor("h_s_aug", [s_len, d + 2], F32R, kind="ExternalInput")
    mb_d = nc.dram_tensor("maskbias", [s_len], F32, kind="ExternalInput")
    w_d = nc.dram_tensor("W_in", [d, d], F32R, kind="ExternalInput")
    id_d = nc.dram_tensor("ident", [128, 128], F32R, kind="ExternalInput")
    id16_d = nc.dram_tensor("ident16", [128, 128], BF16, kind="ExternalInput")
    hsa16_d = nc.dram_tensor("h_s_aug16", [s_len, d + 2], BF16, kind="ExternalInput")
    sc_d = nc.dram_tensor("scores", [t_sh, s_len], F32, kind="ExternalOutput")
    cx_d = nc.dram_tensor("context", [t_sh, d], F32, kind="ExternalOutput")

    with tile.TileContext(nc) as tc:
        with (
            tc.tile_pool(name="persist", bufs=1) as pp,
            tc.tile_pool(name="pblk", bufs=1) as bp,
            tc.tile_pool(name="pout", bufs=2) as op,
            tc.tile_pool(name="work", bufs=2) as wp,
            tc.tile_pool(name="spsum", bufs=2, space="PSUM") as sp,
            tc.tile_pool(name="tpsum", bufs=2, space="PSUM") as tpp,
            tc.tile_pool(name="vpsum", bufs=2, space="PSUM") as vp,
        ):
            # ---- persistent loads ----
            w_sb = pp.tile([128, 2, d], F32R, tag="w")
            nc.sync.dma_start(w_sb[:], w_d.ap().rearrange("(c p) f -> p c f", p=128))
            id_sb = pp.tile([128, 128], F32R, tag="ident")
            nc.sync.dma_start(id_sb[:], id_d.ap())
            id16_sb = pp.tile([128, 128], BF16, tag="ident16")
            nc.sync.dma_start(id16_sb[:], id16_d.ap())
            hsa16_sb = pp.tile([128, nst, d + 2], BF16, tag="hsa16")
            nc.sync.dma_start(
                hsa16_sb[:], hsa16_d.ap().rearrange("(c p) f -> p c f", p=128)
            )
            mb_sb = pp.tile([128, nst], F32, tag="mb")
            nc.sync.dma_start(mb_sb[:], mb_d.ap().rearrange("(c p) -> p c", p=128))
            hsa_sb = pp.tile([128, nst, d + 2], F32R, tag="hsa")
            nc.sync.dma_start(
                hsa_sb[:], hsa_d.ap().rearrange("(c p) f -> p c f", p=128)
            )

            # ---- transposes of h_t and h_s (via PE) ----
            hsT_sb = pp.tile([128, 2, s_len], F32R, tag="hsT")
            gt_sb = pp.tile([128, 2, t_sh], F32R, tag="gt")

            with tc.tile_pool(name="setup", bufs=1) as stp:
                ht_sb = stp.tile([128, nqt, d], F32R, tag="ht")
                nc.sync.dma_start(
                    ht_sb[:], ht_d.ap().rearrange("(c p) f -> p c f", p=128)
                )
                htT_sb = stp.tile([128, 2, t_sh], F32R, tag="htT")
                for tch in range(2):
                    for g in range(0, nqt, 4):
                        n = min(4, nqt - g)
                        ps = tpp.tile([128, 512], F32, tag="tps")
                        for k in range(n):
                            c = g + k
                            nc.tensor.transpose(
                                _r(ps[:, k * 128 : (k + 1) * 128]),
                                ht_sb[:, c, tch * 128 : (tch + 1) * 128],
                                id_sb[:],
                            )
                        nc.vector.tensor_copy(
                            htT_sb[:, tch, g * 128 : (g + n) * 128], ps[:, : n * 128]
                        )
                for dch in range(2):
                    for g in range(0, nst, 4):
                        n = min(4, nst - g)
                        ps = tpp.tile([128, 512], F32, tag="tps")
                        for k in range(n):
                            c = g + k
                            nc.tensor.transpose(
                                _r(ps[:, k * 128 : (k + 1) * 128]),
                                hsa_sb[:, c, dch * 128 : (dch + 1) * 128],
                                id_sb[:],
                            )
                        nc.vector.tensor_copy(
                            hsT_sb[:, dch, g * 128 : (g + n) * 128], ps[:, : n * 128]
                        )

                # ---- G^T = (h_t @ W_in)^T : [d, q] ----
                for dt_ in range(2):
                    for q0 in range(0, t_sh, 512):
                        qn = min(512, t_sh - q0)
                        ps = sp.tile([128, q_blk if q_blk > 512 else 512], F32, tag="sps")
                        for tch in range(2):
                            nc.tensor.matmul(
                                ps[:, :qn],
                                w_sb[:, tch, dt_ * 128 : (dt_ + 1) * 128],
                                htT_sb[:, tch, q0 : q0 + qn],
                                start=(tch == 0),
                                stop=(tch == 1),
                            )
                        nc.vector.tensor_copy(gt_sb[:, dt_, q0 : q0 + qn], ps[:, :qn])

            # ---- main loop over q blocks ----
            for qb in range(nqb):
                q0 = qb * q_blk
                p_blk = bp.tile([128, nst * q_blk], BF16, tag="pblk")
                for st in range(nst):
                    ps = sp.tile([128, q_blk if q_blk > 512 else 512], F32, tag="sps")
                    for dch in range(2):
                        nc.tensor.matmul(
                            ps[:, :q_blk],
                            hsT_sb[:, dch, st * 128 : (st + 1) * 128],
                            gt_sb[:, dch, q0 : q0 + q_blk],
                            start=(dch == 0),
                            stop=(dch == 1),
                        )
                    nc.scalar.activation(
                        p_blk[:, st * q_blk : (st + 1) * q_blk],
                        ps[:, :q_blk],
                        EXP,
                        bias=mb_sb[:, st : st + 1],
                        scale=1.0,
                    )
                for qt in range(qsub):
                    qq = q0 + qt * 128
                    pv = vp.tile([128, d + 2], F32, tag="pv")
                    for st in range(nst):
                        nc.tensor.matmul(
                            pv[:],
                            p_blk[
                                :, st * q_blk + qt * 128 : st * q_blk + (qt + 1) * 128
                            ],
                            hsa16_sb[:, st, :],
                            start=(st == 0),
                            stop=(st == nst - 1),
                        )
                    invz = wp.tile([128, 1], F32, tag="invz")
                    nc.vector.reciprocal(invz[:], pv[:, d : d + 1])
                    cxt = wp.tile([128, d], F32, tag="cxt")
                    nc.vector.tensor_scalar_mul(cxt[:], pv[:, 0:d], invz[:])
                    nc.sync.dma_start(cx_d.ap()[qq : qq + 128, :], cxt[:])

                    pout = op.tile([128, s_len], F32, tag="pout")
                    for sg in range(0, nst, 4):
                        n = min(4, nst - sg)
                        tp = tpp.tile([128, 512], BF16, tag="tps16")
                        for k in range(n):
                            st = sg + k
                            nc.tensor.transpose(
                                tp[:, k * 128 : (k + 1) * 128],
                                p_blk[
                                    :,
                                    st * q_blk
                                    + qt * 128 : st * q_blk
                                    + (qt + 1) * 128,
                                ],
                                id16_sb[:],
                            )
                        nc.vector.tensor_scalar_mul(
                            pout[:, sg * 128 : (sg + n) * 128],
                            tp[:, : n * 128],
                            invz[:],
                        )
                    nc.sync.dma_start(sc_d.ap()[qq : qq + 128, :], pout[:])

    if split_waits:
        _split_multi_waits(nc)
    return nc


_NC_CACHE = {}


def _get_nc():
    if "nc" not in _NC_CACHE:
        _NC_CACHE["nc"] = build_nc()
    return _NC_CACHE["nc"]


def _install_ntff_hook():
    """Dev-only: recreate the missing antenv.axon_hooks NTFF profile hook via
    ctypes on the injected axon .so, so trace=True yields exec_time_ns."""
    import contextlib
    import ctypes
    import sys
    import types

    try:
        import antenv.axon_hooks  # noqa: F401

        return
    except ImportError:
        pass

    so_path = "/opt/axon/libaxon_pjrt.so"
    hook = None
    try:
        lib = ctypes.CDLL(so_path)
        if hasattr(lib, "axon_start_nrt_profile"):
            lib.axon_start_nrt_profile.argtypes = [
                ctypes.POINTER(ctypes.c_int64),
                ctypes.c_size_t,
            ]
            lib.axon_start_nrt_profile.restype = ctypes.c_int64
            lib.axon_stop_nrt_profile.argtypes = [ctypes.c_char_p]
            lib.axon_stop_nrt_profile.restype = ctypes.c_int64

            @contextlib.contextmanager
            def _hook(output_dir, device_ids):
                import jax

                jax.devices()
                if device_ids:
                    ids = (ctypes.c_int64 * len(device_ids))(*device_ids)
                    rc = lib.axon_start_nrt_profile(ids, len(device_ids))
                else:
                    rc = lib.axon_start_nrt_profile(None, 0)
                if rc != 0:
                    raise RuntimeError(f"axon_start_nrt_profile rc={rc}")
                try:
                    yield
                finally:
                    n = lib.axon_stop_nrt_profile(str(output_dir).encode())
                    if n < 0:
                        raise RuntimeError(f"axon_stop_nrt_profile rc={n}")
                    print(f"profile: {n} file(s) written to {output_dir}", file=sys.stderr)

            hook = _hook
    except OSError:
        pass

    mod = types.ModuleType("antenv.axon_hooks")
    mod._hook = hook
    mod.get_axon_ntff_profile_hook = lambda: mod._hook
    mod.set_axon_ntff_profile_hook = lambda h: setattr(mod, "_hook", h)
    sys.modules["antenv.axon_hooks"] = mod
    import antenv

    antenv.axon_hooks = mod

    # artifact upload has no destination in this sandbox
    import concourse.bass_utils as bu

    bu.upload_artifacts = lambda tmpdir: "local://skipped"


def kernel(h_t, h_s, m_s, W_in):
    h_t = np.ascontiguousarray(np.asarray(h_t, dtype=np.float32))
    h_s = np.ascontiguousarray(np.asarray(h_s, dtype=np.float32))
    m_s = np.asarray(m_s)
    W_in = np.ascontiguousarray(np.asarray(W_in, dtype=np.float32))

    nc = _get_nc()
    import ml_dtypes

    ident = np.eye(128, dtype=np.float32)
    ident16 = np.eye(128, dtype=ml_dtypes.bfloat16)
    aug = np.zeros((SRC, 2), dtype=np.float32)
    aug[:, 0] = 1.0
    hsa = [np.ascontiguousarray(np.concatenate([h_s[b], aug], axis=1)) for b in range(B)]
    hsa16 = [np.ascontiguousarray(a.astype(ml_dtypes.bfloat16)) for a in hsa]
    mb = [
        np.ascontiguousarray((m_s[b].astype(np.float32) - 1.0) * NEG_BIG)
        for b in range(B)
    ]

    in_maps = []
    for c in range(N_CORES):
        b, half = c // 2, c % 2
        in_maps.append(
            {
                "h_t": np.ascontiguousarray(
                    h_t[b, half * T_SH : (half + 1) * T_SH, :]
                ),
                "h_s_aug": hsa[b],
                "maskbias": mb[b],
                "W_in": W_in,
                "ident": ident,
                "ident16": ident16,
                "h_s_aug16": hsa16[b],
            }
        )

    trace = os.environ.get("TRNK_TRACE", "0") == "1"
    if trace:
        _install_ntff_hook()
    res = run_bass_kernel_spmd(
        nc, in_maps, core_ids=list(range(N_CORES)), trace=trace
    )
    kernel.last_exec_time_ns = res.exec_time_ns
    kernel.last_results = res

    scores = np.empty((B, TGT, SRC), dtype=np.float32)
    context = np.empty((B, TGT, D), dtype=np.float32)
    for c in range(N_CORES):
        b, half = c // 2, c % 2
        scores[b, half * T_SH : (half + 1) * T_SH, :] = res.results[c]["scores"]
        context[b, half * T_SH : (half + 1) * T_SH, :] = res.results[c]["context"]
    return context, scores


# revision 11
# speedup vs baseline: 1.3138x; 1.0042x over previous
"""Trainium2 Bass kernel for masked Luong-'general' attention.

reference math (per batch b):
    scores = softmax(mask(h_t @ W_in @ h_s^T))       # [T, S]
    context = scores @ h_s                           # [T, D]
returns (context, scores).

Sharding: 8 cores = 4 batches x 2 halves of the target sequence.
Each core computes a [2048, 4096] score block + [2048, 256] context block.

Per-core dataflow (all matmuls in float32r = fp22, full PE rate):
  G^T[d,q]   = W_in(native lhsT) . h_tT          (h_tT via PE transposes)
  S_T[s,q]   = h_sT-chunks(lhsT) . G^T           (PSUM)
  P_T[s,q]   = exp(S_T + maskbias[s])            (one ACT op: mask+exp+drain)
  ctx|Z[q,.] = P_T-chunks(lhsT) . [h_s | 1]      (ones col gives softmax denom)
  scores     = transpose(P_T) * (1/Z)            (PE transpose + DVE drain)
"""

import os
import numpy as np

import concourse.bass as bass
import concourse.tile as tile
import concourse.mybir as mybir
from concourse.bass_utils import run_bass_kernel_spmd

F32 = mybir.dt.float32
F32R = mybir.dt.float32r
BF16 = mybir.dt.bfloat16
EXP = mybir.ActivationFunctionType.Exp

N_CORES = 8
B, TGT, SRC, D = 4, 4096, 4096, 256
T_SH = TGT * B // N_CORES  # 2048 target rows per core
NEG_BIG = 1.0e11


def _r(ap):
    return ap.bitcast(F32R)


def _split_multi_waits(nc, max_waits=1):
    """This walrus build's CTRL templates (Drain etc.) only accept one sync
    wait; peel extra waits onto preceding NoOps on the same engine."""
    for f in nc.m.functions:
        for bb in f.blocks:
            insts = bb.instructions
            i = 0
            while i < len(insts):
                inst = insts[i]
                si = inst.sync_info
                if si is not None and len(si.on_wait) > max_waits:
                    waits = list(si.on_wait)
                    keep = waits[-max_waits:]
                    extra = waits[:-max_waits]
                    nops = []
                    for j in range(0, len(extra), max_waits):
                        nops.append(
                            mybir.InstNoOp(
                                name=f"{inst.name}_ws{j}",
                                engine=inst.engine,
                                bass_nofuse=True,
                                sync_info=mybir.SyncInfo(
                                    on_wait=extra[j : j + max_waits], on_update=[]
                                ),
                            )
                        )
                    inst.sync_info = mybir.SyncInfo(
                        on_wait=keep, on_update=list(si.on_update)
                    )
                    for k, nop in enumerate(nops):
                        insts.insert(i + k, nop)
                    i += len(nops)
                i += 1


def build_nc(t_sh=T_SH, s_len=SRC, q_blk=512, split_waits=True):
    """Build the per-core SPMD program. All cores run the same NEFF."""
    d = D
    nqt = t_sh // 128  # q tiles (of 128)
    nst = s_len // 128  # s tiles
    nqb = t_sh // q_blk  # q blocks held in SBUF at once
    qsub = q_blk // 128  # q subtiles per block

    nc = bass.Bass(trn_type="TRN2", target_bir_lowering=False, debug=False)

    ht_d = nc.dram_tensor("h_t", [t_sh, d], F32R, kind="ExternalInput")
    hsa_d = nc.dram_tensor("h_s_aug", [s_len, d + 2], F32R, kind="ExternalInput")
    mb_d = nc.dram_tensor("maskbias", [s_len], F32, kind="ExternalInput")
    w_d = nc.dram_tensor("W_in", [d, d], F32R, kind="ExternalInput")
    id_d = nc.dram_tensor("ident", [128, 128], F32R, kind="ExternalInput")
    id16_d = nc.dram_tensor("ident16", [128, 128], BF16, kind="ExternalInput")
    hsa16_d = nc.dram_tensor("h_s_aug16", [s_len, d + 2], BF16, kind="ExternalInput")
    sc_d = nc.dram_tensor("scores", [t_sh, s_len], F32, kind="ExternalOutput")
    cx_d = nc.dram_tensor("context", [t_sh, d], F32, kind="ExternalOutput")

    with tile.TileContext(nc) as tc:
        with (
            tc.tile_pool(name="persist", bufs=1) as pp,
            tc.tile_pool(name="pblk", bufs=1) as bp,
            tc.tile_pool(name="pout", bufs=2) as op,
            tc.tile_pool(name="work", bufs=2) as wp,
            tc.tile_pool(name="spsum", bufs=2, space="PSUM") as sp,
            tc.tile_pool(name="tpsum", bufs=2, space="PSUM") as tpp,
            tc.tile_pool(name="vpsum", bufs=2, space="PSUM") as vp,
        ):
            # ---- persistent loads ----
            w_sb = pp.tile([128, 2, d], F32R, tag="w")
            nc.sync.dma_start(w_sb[:], w_d.ap().rearrange("(c p) f -> p c f", p=128))
            id_sb = pp.tile([128, 128], F32R, tag="ident")
            nc.sync.dma_start(id_sb[:], id_d.ap())
            id16_sb = pp.tile([128, 128], BF16, tag="ident16")
            nc.sync.dma_start(id16_sb[:], id16_d.ap())
            hsa16_sb = pp.tile([128, nst, d + 2], BF16, tag="hsa16")
            nc.sync.dma_start(
                hsa16_sb[:], hsa16_d.ap().rearrange("(c p) f -> p c f", p=128)
            )
            mb_sb = pp.tile([128, nst], F32, tag="mb")
            nc.sync.dma_start(mb_sb[:], mb_d.ap().rearrange("(c p) -> p c", p=128))
            hsa_sb = pp.tile([128, nst, d + 2], F32R, tag="hsa")
            nc.sync.dma_start(
                hsa_sb[:], hsa_d.ap().rearrange("(c p) f -> p c f", p=128)
            )

            # ---- transposes of h_t and h_s (via PE) ----
            hsT_sb = pp.tile([128, 2, s_len], F32R, tag="hsT")
            gt_sb = pp.tile([128, 2, t_sh], F32R, tag="gt")

            with tc.tile_pool(name="setup", bufs=1) as stp:
                ht_sb = stp.tile([128, nqt, d], F32R, tag="ht")
                nc.sync.dma_start(
                    ht_sb[:], ht_d.ap().rearrange("(c p) f -> p c f", p=128)
                )
                htT_sb = stp.tile([128, 2, t_sh], F32R, tag="htT")
                for tch in range(2):
                    for g in range(0, nqt, 4):
                        n = min(4, nqt - g)
                        ps = tpp.tile([128, 512], F32, tag="tps")
                        for k in range(n):
                            c = g + k
                            nc.tensor.transpose(
                                _r(ps[:, k * 128 : (k + 1) * 128]),
                                ht_sb[:, c, tch * 128 : (tch + 1) * 128],
                                id_sb[:],
                            )
                        nc.vector.tensor_copy(
                            htT_sb[:, tch, g * 128 : (g + n) * 128], ps[:, : n * 128]
                        )
                for dch in range(2):
                    for g in range(0, nst, 4):
                        n = min(4, nst - g)
                        ps = tpp.tile([128, 512], F32, tag="tps")
                        for k in range(n):
                            c = g + k
                            nc.tensor.transpose(
                                _r(ps[:, k * 128 : (k + 1) * 128]),
                                hsa_sb[:, c, dch * 128 : (dch + 1) * 128],
                                id_sb[:],
                            )
                        nc.vector.tensor_copy(
                            hsT_sb[:, dch, g * 128 : (g + n) * 128], ps[:, : n * 128]
                        )

                # ---- G^T = (h_t @ W_in)^T : [d, q] ----
                for dt_ in range(2):
                    for q0 in range(0, t_sh, 512):
                        qn = min(512, t_sh - q0)
                        ps = sp.tile([128, q_blk if q_blk > 512 else 512], F32, tag="sps")
                        for tch in range(2):
                            nc.tensor.matmul(
                                ps[:, :qn],
                                w_sb[:, tch, dt_ * 128 : (dt_ + 1) * 128],
                                htT_sb[:, tch, q0 : q0 + qn],
                                start=(tch == 0),
                                stop=(tch == 1),
                            )
                        nc.vector.tensor_copy(gt_sb[:, dt_, q0 : q0 + qn], ps[:, :qn])

            # ---- main loop over q blocks ----
            for qb in range(nqb):
                q0 = qb * q_blk
                p_blk = bp.tile([128, nst * q_blk], BF16, tag="pblk")
                for st in range(nst):
                    ps = sp.tile([128, q_blk if q_blk > 512 else 512], F32, tag="sps")
                    for dch in range(2):
                        nc.tensor.matmul(
                            ps[:, :q_blk],
                            hsT_sb[:, dch, st * 128 : (st + 1) * 128],
                            gt_sb[:, dch, q0 : q0 + q_blk],
                            start=(dch == 0),
                            stop=(dch == 1),
                        )
                    nc.scalar.activation(
                        p_blk[:, st * q_blk : (st + 1) * q_blk],
                        ps[:, :q_blk],
                        EXP,
                        bias=mb_sb[:, st : st + 1],
                        scale=1.0,
                    )
                for qt in range(qsub):
                    qq = q0 + qt * 128
                    pv = vp.tile([128, d + 2], F32, tag="pv")
                    for st in range(nst):
                        nc.tensor.matmul(
                            pv[:],
                            p_blk[
                                :, st * q_blk + qt * 128 : st * q_blk + (qt + 1) * 128
                            ],
                            hsa16_sb[:, st, :],
                            start=(st == 0),
                            stop=(st == nst - 1),
                        )
                    invz = wp.tile([128, 1], F32, tag="invz")
                    nc.vector.reciprocal(invz[:], pv[:, d : d + 1])
                    cxt = wp.tile([128, d], F32, tag="cxt")
                    nc.vector.tensor_scalar_mul(cxt[:], pv[:, 0:d], invz[:])
                    nc.sync.dma_start(cx_d.ap()[qq : qq + 128, :], cxt[:])

                    pout = op.tile([128, s_len], BF16, tag="pout")
                    for sg in range(0, nst, 4):
                        n = min(4, nst - sg)
                        tp = tpp.tile([128, 512], BF16, tag="tps16")
                        for k in range(n):
                            st = sg + k
                            nc.tensor.transpose(
                                tp[:, k * 128 : (k + 1) * 128],
                                p_blk[
                                    :,
                                    st * q_blk
                                    + qt * 128 : st * q_blk
                                    + (qt + 1) * 128,
                                ],
                                id16_sb[:],
                            )
                        nc.vector.tensor_scalar_mul(
                            pout[:, sg * 128 : (sg + n) * 128],
                            tp[:, : n * 128],
                            invz[:],
                        )
                    nc.gpsimd.dma_start(sc_d.ap()[qq : qq + 128, :], pout[:])

    if split_waits:
        _split_multi_waits(nc)
    return nc


_NC_CACHE = {}


def _get_nc():
    if "nc" not in _NC_CACHE:
        _NC_CACHE["nc"] = build_nc()
    return _NC_CACHE["nc"]


def _install_ntff_hook():
    """Dev-only: recreate the missing antenv.axon_hooks NTFF profile hook via
    ctypes on the injected axon .so, so trace=True yields exec_time_ns."""
    import contextlib
    import ctypes
    import sys
    import types

    try:
        import antenv.axon_hooks  # noqa: F401

        return
    except ImportError:
        pass

    so_path = "/opt/axon/libaxon_pjrt.so"
    hook = None
    try:
        lib = ctypes.CDLL(so_path)
        if hasattr(lib, "axon_start_nrt_profile"):
            lib.axon_start_nrt_profile.argtypes = [
                ctypes.POINTER(ctypes.c_int64),
                ctypes.c_size_t,
            ]
            lib.axon_start_nrt_profile.restype = ctypes.c_int64
            lib.axon_stop_nrt_profile.argtypes = [ctypes.c_char_p]
            lib.axon_stop_nrt_profile.restype = ctypes.c_int64

            @contextlib.contextmanager
            def _hook(output_dir, device_ids):
                import jax

                jax.devices()
                if device_ids:
                    ids = (ctypes.c_int64 * len(device_ids))(*device_ids)
                    rc = lib.axon_start_nrt_profile(ids, len(device_ids))
                else:
                    rc = lib.axon_start_nrt_profile(None, 0)
                if rc != 0:
                    raise RuntimeError(f"axon_start_nrt_profile rc={rc}")
                try:
                    yield
                finally:
                    n = lib.axon_stop_nrt_profile(str(output_dir).encode())
                    if n < 0:
                        raise RuntimeError(f"axon_stop_nrt_profile rc={n}")
                    print(f"profile: {n} file(s) written to {output_dir}", file=sys.stderr)

            hook = _hook
    except OSError:
        pass

    mod = types.ModuleType("antenv.axon_hooks")
    mod._hook = hook
    mod.get_axon_ntff_profile_hook = lambda: mod._hook
    mod.set_axon_ntff_profile_hook = lambda h: setattr(mod, "_hook", h)
    sys.modules["antenv.axon_hooks"] = mod
    import antenv

    antenv.axon_hooks = mod

    # artifact upload has no destination in this sandbox
    import concourse.bass_utils as bu

    bu.upload_artifacts = lambda tmpdir: "local://skipped"


def kernel(h_t, h_s, m_s, W_in):
    h_t = np.ascontiguousarray(np.asarray(h_t, dtype=np.float32))
    h_s = np.ascontiguousarray(np.asarray(h_s, dtype=np.float32))
    m_s = np.asarray(m_s)
    W_in = np.ascontiguousarray(np.asarray(W_in, dtype=np.float32))

    nc = _get_nc()
    import ml_dtypes

    ident = np.eye(128, dtype=np.float32)
    ident16 = np.eye(128, dtype=ml_dtypes.bfloat16)
    aug = np.zeros((SRC, 2), dtype=np.float32)
    aug[:, 0] = 1.0
    hsa = [np.ascontiguousarray(np.concatenate([h_s[b], aug], axis=1)) for b in range(B)]
    hsa16 = [np.ascontiguousarray(a.astype(ml_dtypes.bfloat16)) for a in hsa]
    mb = [
        np.ascontiguousarray((m_s[b].astype(np.float32) - 1.0) * NEG_BIG)
        for b in range(B)
    ]

    in_maps = []
    for c in range(N_CORES):
        b, half = c // 2, c % 2
        in_maps.append(
            {
                "h_t": np.ascontiguousarray(
                    h_t[b, half * T_SH : (half + 1) * T_SH, :]
                ),
                "h_s_aug": hsa[b],
                "maskbias": mb[b],
                "W_in": W_in,
                "ident": ident,
                "ident16": ident16,
                "h_s_aug16": hsa16[b],
            }
        )

    trace = os.environ.get("TRNK_TRACE", "0") == "1"
    if trace:
        _install_ntff_hook()
    res = run_bass_kernel_spmd(
        nc, in_maps, core_ids=list(range(N_CORES)), trace=trace
    )
    kernel.last_exec_time_ns = res.exec_time_ns
    kernel.last_results = res

    scores = np.empty((B, TGT, SRC), dtype=np.float32)
    context = np.empty((B, TGT, D), dtype=np.float32)
    for c in range(N_CORES):
        b, half = c // 2, c % 2
        scores[b, half * T_SH : (half + 1) * T_SH, :] = res.results[c]["scores"]
        context[b, half * T_SH : (half + 1) * T_SH, :] = res.results[c]["context"]
    return context, scores


# revision 12
# speedup vs baseline: 1.3988x; 1.0647x over previous
"""Trainium2 Bass kernel for masked Luong-'general' attention.

reference math (per batch b):
    scores = softmax(mask(h_t @ W_in @ h_s^T))       # [T, S]
    context = scores @ h_s                           # [T, D]
returns (context, scores).

Sharding: 8 cores = 4 batches x 2 halves of the target sequence.
Each core computes a [2048, 4096] score block + [2048, 256] context block.

Per-core dataflow (all matmuls in float32r = fp22, full PE rate):
  G^T[d,q]   = W_in(native lhsT) . h_tT          (h_tT via PE transposes)
  S_T[s,q]   = h_sT-chunks(lhsT) . G^T           (PSUM)
  P_T[s,q]   = exp(S_T + maskbias[s])            (one ACT op: mask+exp+drain)
  ctx|Z[q,.] = P_T-chunks(lhsT) . [h_s | 1]      (ones col gives softmax denom)
  scores     = transpose(P_T) * (1/Z)            (PE transpose + DVE drain)
"""

import os
import numpy as np

import concourse.bass as bass
import concourse.tile as tile
import concourse.mybir as mybir
from concourse.bass_utils import run_bass_kernel_spmd

F32 = mybir.dt.float32
F32R = mybir.dt.float32r
BF16 = mybir.dt.bfloat16
EXP = mybir.ActivationFunctionType.Exp

N_CORES = 8
B, TGT, SRC, D = 4, 4096, 4096, 256
T_SH = TGT * B // N_CORES  # 2048 target rows per core
NEG_BIG = 1.0e11


def _r(ap):
    return ap.bitcast(F32R)


def _split_multi_waits(nc, max_waits=1):
    """This walrus build's CTRL templates (Drain etc.) only accept one sync
    wait; peel extra waits onto preceding NoOps on the same engine."""
    for f in nc.m.functions:
        for bb in f.blocks:
            insts = bb.instructions
            i = 0
            while i < len(insts):
                inst = insts[i]
                si = inst.sync_info
                if si is not None and len(si.on_wait) > max_waits:
                    waits = list(si.on_wait)
                    keep = waits[-max_waits:]
                    extra = waits[:-max_waits]
                    nops = []
                    for j in range(0, len(extra), max_waits):
                        nops.append(
                            mybir.InstNoOp(
                                name=f"{inst.name}_ws{j}",
                                engine=inst.engine,
                                bass_nofuse=True,
                                sync_info=mybir.SyncInfo(
                                    on_wait=extra[j : j + max_waits], on_update=[]
                                ),
                            )
                        )
                    inst.sync_info = mybir.SyncInfo(
                        on_wait=keep, on_update=list(si.on_update)
                    )
                    for k, nop in enumerate(nops):
                        insts.insert(i + k, nop)
                    i += len(nops)
                i += 1


def build_nc(t_sh=T_SH, s_len=SRC, q_blk=512, split_waits=True):
    """Build the per-core SPMD program. All cores run the same NEFF."""
    d = D
    nqt = t_sh // 128  # q tiles (of 128)
    nst = s_len // 128  # s tiles
    nqb = t_sh // q_blk  # q blocks held in SBUF at once
    qsub = q_blk // 128  # q subtiles per block

    nc = bass.Bass(trn_type="TRN2", target_bir_lowering=False, debug=False)

    mb_d = nc.dram_tensor("maskbias", [s_len], F32, kind="ExternalInput")
    w_d = nc.dram_tensor("W_in", [d, d], F32R, kind="ExternalInput")
    id16_d = nc.dram_tensor("ident16", [128, 128], BF16, kind="ExternalInput")
    hsa16_d = nc.dram_tensor("h_s_aug16", [s_len, d + 2], BF16, kind="ExternalInput")
    hs_hi_d = nc.dram_tensor("hs_hi", [s_len, d], BF16, kind="ExternalInput")
    hs_lo_d = nc.dram_tensor("hs_lo", [s_len, d], BF16, kind="ExternalInput")
    ht_hi_d = nc.dram_tensor("ht_hi", [t_sh, d], BF16, kind="ExternalInput")
    ht_lo_d = nc.dram_tensor("ht_lo", [t_sh, d], BF16, kind="ExternalInput")
    sc_d = nc.dram_tensor("scores", [t_sh, s_len], F32, kind="ExternalOutput")
    cx_d = nc.dram_tensor("context", [t_sh, d], F32, kind="ExternalOutput")

    with tile.TileContext(nc) as tc:
        with (
            tc.tile_pool(name="persist", bufs=1) as pp,
            tc.tile_pool(name="pblk", bufs=2) as bp,
            tc.tile_pool(name="pout", bufs=2) as op,
            tc.tile_pool(name="work", bufs=2) as wp,
            tc.tile_pool(name="spsum", bufs=2, space="PSUM") as sp,
            tc.tile_pool(name="tpsum", bufs=2, space="PSUM") as tpp,
            tc.tile_pool(name="vpsum", bufs=2, space="PSUM") as vp,
        ):
            # ---- persistent loads ----
            w_sb = pp.tile([128, 2, d], F32R, tag="w")
            nc.sync.dma_start(w_sb[:], w_d.ap().rearrange("(c p) f -> p c f", p=128))
            id16_sb = pp.tile([128, 128], BF16, tag="ident16")
            nc.sync.dma_start(id16_sb[:], id16_d.ap())
            hsa16_sb = pp.tile([128, nst, d + 2], BF16, tag="hsa16")
            nc.sync.dma_start(
                hsa16_sb[:], hsa16_d.ap().rearrange("(c p) f -> p c f", p=128)
            )
            mb_sb = pp.tile([128, nst], F32, tag="mb")
            nc.sync.dma_start(mb_sb[:], mb_d.ap().rearrange("(c p) -> p c", p=128))

            # ---- transposes of h_t and h_s (via PE) ----
            hsT_sb = pp.tile([128, 2, s_len], F32R, tag="hsT")
            gt_sb = pp.tile([128, 2, t_sh], F32R, tag="gt")

            with tc.tile_pool(name="setup", bufs=1) as stp:
                htT_sb = stp.tile([128, 2, t_sh], F32R, tag="htT")
                for dch in range(2):
                    hiT = stp.tile([128, s_len if s_len >= t_sh else t_sh], BF16, tag="hiT")
                    loT = stp.tile([128, s_len if s_len >= t_sh else t_sh], BF16, tag="loT")
                    nc.sync.dma_start(
                        hiT[:, :t_sh],
                        ht_hi_d.ap()[:, dch * 128 : (dch + 1) * 128],
                        transpose=True,
                    )
                    nc.sync.dma_start(
                        loT[:, :t_sh],
                        ht_lo_d.ap()[:, dch * 128 : (dch + 1) * 128],
                        transpose=True,
                    )
                    nc.vector.tensor_add(htT_sb[:, dch, :], hiT[:, :t_sh], loT[:, :t_sh])
                for dch in range(2):
                    hiT = stp.tile([128, s_len if s_len >= t_sh else t_sh], BF16, tag="hiT")
                    loT = stp.tile([128, s_len if s_len >= t_sh else t_sh], BF16, tag="loT")
                    nc.sync.dma_start(
                        hiT[:, :s_len],
                        hs_hi_d.ap()[:, dch * 128 : (dch + 1) * 128],
                        transpose=True,
                    )
                    nc.sync.dma_start(
                        loT[:, :s_len],
                        hs_lo_d.ap()[:, dch * 128 : (dch + 1) * 128],
                        transpose=True,
                    )
                    nc.vector.tensor_add(hsT_sb[:, dch, :], hiT[:, :s_len], loT[:, :s_len])

                # ---- G^T = (h_t @ W_in)^T : [d, q] ----
                for dt_ in range(2):
                    for q0 in range(0, t_sh, 512):
                        qn = min(512, t_sh - q0)
                        ps = sp.tile([128, q_blk if q_blk > 512 else 512], F32, tag="sps")
                        for tch in range(2):
                            nc.tensor.matmul(
                                ps[:, :qn],
                                w_sb[:, tch, dt_ * 128 : (dt_ + 1) * 128],
                                htT_sb[:, tch, q0 : q0 + qn],
                                start=(tch == 0),
                                stop=(tch == 1),
                            )
                        nc.vector.tensor_copy(gt_sb[:, dt_, q0 : q0 + qn], ps[:, :qn])

            # ---- main loop over q blocks ----
            for qb in range(nqb):
                q0 = qb * q_blk
                p_blk = bp.tile([128, nst * q_blk], BF16, tag="pblk")
                for st in range(nst):
                    ps = sp.tile([128, q_blk if q_blk > 512 else 512], F32, tag="sps")
                    for dch in range(2):
                        nc.tensor.matmul(
                            ps[:, :q_blk],
                            hsT_sb[:, dch, st * 128 : (st + 1) * 128],
                            gt_sb[:, dch, q0 : q0 + q_blk],
                            start=(dch == 0),
                            stop=(dch == 1),
                        )
                    nc.scalar.activation(
                        p_blk[:, st * q_blk : (st + 1) * q_blk],
                        ps[:, :q_blk],
                        EXP,
                        bias=mb_sb[:, st : st + 1],
                        scale=1.0,
                    )
                for qt in range(qsub):
                    qq = q0 + qt * 128
                    pv = vp.tile([128, d + 2], F32, tag="pv")
                    for st in range(nst):
                        nc.tensor.matmul(
                            pv[:],
                            p_blk[
                                :, st * q_blk + qt * 128 : st * q_blk + (qt + 1) * 128
                            ],
                            hsa16_sb[:, st, :],
                            start=(st == 0),
                            stop=(st == nst - 1),
                        )
                    invz = wp.tile([128, 1], F32, tag="invz")
                    nc.vector.reciprocal(invz[:], pv[:, d : d + 1])
                    cxt = wp.tile([128, d], F32, tag="cxt")
                    nc.vector.tensor_scalar_mul(cxt[:], pv[:, 0:d], invz[:])
                    nc.sync.dma_start(cx_d.ap()[qq : qq + 128, :], cxt[:])

                    pout = op.tile([128, s_len], BF16, tag="pout")
                    for sg in range(0, nst, 4):
                        n = min(4, nst - sg)
                        tp = tpp.tile([128, 512], BF16, tag="tps16")
                        for k in range(n):
                            st = sg + k
                            nc.tensor.transpose(
                                tp[:, k * 128 : (k + 1) * 128],
                                p_blk[
                                    :,
                                    st * q_blk
                                    + qt * 128 : st * q_blk
                                    + (qt + 1) * 128,
                                ],
                                id16_sb[:],
                            )
                        nc.vector.tensor_scalar_mul(
                            pout[:, sg * 128 : (sg + n) * 128],
                            tp[:, : n * 128],
                            invz[:],
                        )
                    nc.gpsimd.dma_start(sc_d.ap()[qq : qq + 128, :], pout[:])

    if split_waits:
        _split_multi_waits(nc)
    return nc


_NC_CACHE = {}


def _get_nc():
    if "nc" not in _NC_CACHE:
        _NC_CACHE["nc"] = build_nc()
    return _NC_CACHE["nc"]


def _install_ntff_hook():
    """Dev-only: recreate the missing antenv.axon_hooks NTFF profile hook via
    ctypes on the injected axon .so, so trace=True yields exec_time_ns."""
    import contextlib
    import ctypes
    import sys
    import types

    try:
        import antenv.axon_hooks  # noqa: F401

        return
    except ImportError:
        pass

    so_path = "/opt/axon/libaxon_pjrt.so"
    hook = None
    try:
        lib = ctypes.CDLL(so_path)
        if hasattr(lib, "axon_start_nrt_profile"):
            lib.axon_start_nrt_profile.argtypes = [
                ctypes.POINTER(ctypes.c_int64),
                ctypes.c_size_t,
            ]
            lib.axon_start_nrt_profile.restype = ctypes.c_int64
            lib.axon_stop_nrt_profile.argtypes = [ctypes.c_char_p]
            lib.axon_stop_nrt_profile.restype = ctypes.c_int64

            @contextlib.contextmanager
            def _hook(output_dir, device_ids):
                import jax

                jax.devices()
                if device_ids:
                    ids = (ctypes.c_int64 * len(device_ids))(*device_ids)
                    rc = lib.axon_start_nrt_profile(ids, len(device_ids))
                else:
                    rc = lib.axon_start_nrt_profile(None, 0)
                if rc != 0:
                    raise RuntimeError(f"axon_start_nrt_profile rc={rc}")
                try:
                    yield
                finally:
                    n = lib.axon_stop_nrt_profile(str(output_dir).encode())
                    if n < 0:
                        raise RuntimeError(f"axon_stop_nrt_profile rc={n}")
                    print(f"profile: {n} file(s) written to {output_dir}", file=sys.stderr)

            hook = _hook
    except OSError:
        pass

    mod = types.ModuleType("antenv.axon_hooks")
    mod._hook = hook
    mod.get_axon_ntff_profile_hook = lambda: mod._hook
    mod.set_axon_ntff_profile_hook = lambda h: setattr(mod, "_hook", h)
    sys.modules["antenv.axon_hooks"] = mod
    import antenv

    antenv.axon_hooks = mod

    # artifact upload has no destination in this sandbox
    import concourse.bass_utils as bu

    bu.upload_artifacts = lambda tmpdir: "local://skipped"


def kernel(h_t, h_s, m_s, W_in):
    h_t = np.ascontiguousarray(np.asarray(h_t, dtype=np.float32))
    h_s = np.ascontiguousarray(np.asarray(h_s, dtype=np.float32))
    m_s = np.asarray(m_s)
    W_in = np.ascontiguousarray(np.asarray(W_in, dtype=np.float32))

    nc = _get_nc()
    import ml_dtypes

    ident16 = np.eye(128, dtype=ml_dtypes.bfloat16)

    def hilo(x):
        hi = x.astype(ml_dtypes.bfloat16)
        lo = (x - hi.astype(np.float32)).astype(ml_dtypes.bfloat16)
        return np.ascontiguousarray(hi), np.ascontiguousarray(lo)
    aug = np.zeros((SRC, 2), dtype=np.float32)
    aug[:, 0] = 1.0
    hsa = [np.ascontiguousarray(np.concatenate([h_s[b], aug], axis=1)) for b in range(B)]
    hsa16 = [np.ascontiguousarray(a.astype(ml_dtypes.bfloat16)) for a in hsa]
    hs_hilo = [hilo(h_s[b]) for b in range(B)]
    mb = [
        np.ascontiguousarray((m_s[b].astype(np.float32) - 1.0) * NEG_BIG)
        for b in range(B)
    ]

    in_maps = []
    for c in range(N_CORES):
        b, half = c // 2, c % 2
        ht_hi, ht_lo = hilo(h_t[b, half * T_SH : (half + 1) * T_SH, :])
        in_maps.append(
            {
                "maskbias": mb[b],
                "W_in": W_in,
                "ident16": ident16,
                "h_s_aug16": hsa16[b],
                "hs_hi": hs_hilo[b][0],
                "hs_lo": hs_hilo[b][1],
                "ht_hi": ht_hi,
                "ht_lo": ht_lo,
            }
        )

    trace = os.environ.get("TRNK_TRACE", "0") == "1"
    if trace:
        _install_ntff_hook()
    res = run_bass_kernel_spmd(
        nc, in_maps, core_ids=list(range(N_CORES)), trace=trace
    )
    kernel.last_exec_time_ns = res.exec_time_ns
    kernel.last_results = res

    scores = np.empty((B, TGT, SRC), dtype=np.float32)
    context = np.empty((B, TGT, D), dtype=np.float32)
    for c in range(N_CORES):
        b, half = c // 2, c % 2
        scores[b, half * T_SH : (half + 1) * T_SH, :] = res.results[c]["scores"]
        context[b, half * T_SH : (half + 1) * T_SH, :] = res.results[c]["context"]
    return context, scores


# revision 17
# speedup vs baseline: 1.4625x; 1.0455x over previous
"""Trainium2 Bass kernel for masked Luong-'general' attention.

reference math (per batch b):
    scores = softmax(mask(h_t @ W_in @ h_s^T))       # [T, S]
    context = scores @ h_s                           # [T, D]
returns (context, scores).

Sharding: 8 cores = 4 batches x 2 halves of the target sequence.
Each core computes a [2048, 4096] score block + [2048, 256] context block.

Per-core dataflow (all matmuls in float32r = fp22, full PE rate):
  G^T[d,q]   = W_in(native lhsT) . h_tT          (h_tT via PE transposes)
  S_T[s,q]   = h_sT-chunks(lhsT) . G^T           (PSUM)
  P_T[s,q]   = exp(S_T + maskbias[s])            (one ACT op: mask+exp+drain)
  ctx|Z[q,.] = P_T-chunks(lhsT) . [h_s | 1]      (ones col gives softmax denom)
  scores     = transpose(P_T) * (1/Z)            (PE transpose + DVE drain)
"""

import os
import numpy as np

import concourse.bass as bass
import concourse.tile as tile
import concourse.mybir as mybir
from concourse.bass_utils import run_bass_kernel_spmd

F32 = mybir.dt.float32
F32R = mybir.dt.float32r
BF16 = mybir.dt.bfloat16
EXP = mybir.ActivationFunctionType.Exp

N_CORES = 8
B, TGT, SRC, D = 4, 4096, 4096, 256
T_SH = TGT * B // N_CORES  # 2048 target rows per core
NEG_BIG = 1.0e11


def _r(ap):
    return ap.bitcast(F32R)


def _split_multi_waits(nc, max_waits=1):
    """This walrus build's CTRL templates (Drain etc.) only accept one sync
    wait; peel extra waits onto preceding NoOps on the same engine."""
    for f in nc.m.functions:
        for bb in f.blocks:
            insts = bb.instructions
            i = 0
            while i < len(insts):
                inst = insts[i]
                si = inst.sync_info
                if si is not None and len(si.on_wait) > max_waits:
                    waits = list(si.on_wait)
                    keep = waits[-max_waits:]
                    extra = waits[:-max_waits]
                    nops = []
                    for j in range(0, len(extra), max_waits):
                        nops.append(
                            mybir.InstNoOp(
                                name=f"{inst.name}_ws{j}",
                                engine=inst.engine,
                                bass_nofuse=True,
                                sync_info=mybir.SyncInfo(
                                    on_wait=extra[j : j + max_waits], on_update=[]
                                ),
                            )
                        )
                    inst.sync_info = mybir.SyncInfo(
                        on_wait=keep, on_update=list(si.on_update)
                    )
                    for k, nop in enumerate(nops):
                        insts.insert(i + k, nop)
                    i += len(nops)
                i += 1


def build_nc(t_sh=T_SH, s_len=SRC, q_blk=512, split_waits=True):
    """Build the per-core SPMD program. All cores run the same NEFF."""
    d = D
    nst = s_len // 128  # s tiles
    nqb = t_sh // q_blk  # q blocks held in SBUF at once
    qsub = q_blk // 128  # q subtiles per block

    nc = bass.Bass(trn_type="TRN2", target_bir_lowering=False, debug=False)

    mb_d = nc.dram_tensor("maskbias", [s_len], F32, kind="ExternalInput")
    w_d = nc.dram_tensor("W_in", [d, d], F32R, kind="ExternalInput")
    id16_d = nc.dram_tensor("ident16", [128, 128], BF16, kind="ExternalInput")
    hsa16_d = nc.dram_tensor("h_s_aug16", [s_len, d + 2], BF16, kind="ExternalInput")
    hs_hi_d = nc.dram_tensor("hs_hi", [s_len, d], BF16, kind="ExternalInput")
    hs_lo_d = nc.dram_tensor("hs_lo", [s_len, d], BF16, kind="ExternalInput")
    ht_hi_d = nc.dram_tensor("ht_hi", [t_sh, d], BF16, kind="ExternalInput")
    ht_lo_d = nc.dram_tensor("ht_lo", [t_sh, d], BF16, kind="ExternalInput")
    sc_d = nc.dram_tensor("scores", [t_sh, s_len], F32, kind="ExternalOutput")
    cx_d = nc.dram_tensor("context", [t_sh, d], F32, kind="ExternalOutput")

    with tile.TileContext(nc) as tc:
        with (
            tc.tile_pool(name="persist", bufs=1) as pp,
            tc.tile_pool(name="pblk", bufs=2) as bp,
            tc.tile_pool(name="pout", bufs=2) as op,
            tc.tile_pool(name="work", bufs=2) as wp,
            tc.tile_pool(name="spsum", bufs=3, space="PSUM") as sp,
            tc.tile_pool(name="tpsum", bufs=2, space="PSUM") as tpp,
            tc.tile_pool(name="vpsum", bufs=2, space="PSUM") as vp,
        ):
            # ---- persistent tiles (v4 allocation order) ----
            w_sb = pp.tile([128, 2, d], F32R, tag="w")
            nc.sync.dma_start(w_sb[:], w_d.ap().rearrange("(c p) f -> p c f", p=128))
            id16_sb = pp.tile([128, 128], BF16, tag="ident16")
            nc.sync.dma_start(id16_sb[:], id16_d.ap())
            hsa16_sb = pp.tile([128, nst, d + 2], BF16, tag="hsa16")
            nc.sync.dma_start(
                hsa16_sb[:], hsa16_d.ap().rearrange("(c p) f -> p c f", p=128)
            )
            mb_sb = pp.tile([128, nst], F32, tag="mb")
            nc.sync.dma_start(mb_sb[:], mb_d.ap().rearrange("(c p) -> p c", p=128))
            hsT_sb = pp.tile([128, 2, s_len], F32R, tag="hsT")
            gt_sb = pp.tile([128, 2, t_sh], F32R, tag="gt")

            # ---- transposed input loads (xbar DMA, hi/lo bf16 split) ----
            with tc.tile_pool(name="setup", bufs=1) as stp:
                htT_sb = stp.tile([128, 2, t_sh], F32R, tag="htT")
                big = s_len if s_len >= t_sh else t_sh
                for dch in range(2):
                    hiT = stp.tile([128, big], BF16, tag="hiT")
                    loT = stp.tile([128, big], BF16, tag="loT")
                    nc.sync.dma_start(
                        hiT[:, :t_sh],
                        ht_hi_d.ap()[:, dch * 128 : (dch + 1) * 128],
                        transpose=True,
                    )
                    nc.sync.dma_start(
                        loT[:, :t_sh],
                        ht_lo_d.ap()[:, dch * 128 : (dch + 1) * 128],
                        transpose=True,
                    )
                    nc.vector.tensor_add(
                        htT_sb[:, dch, :], hiT[:, :t_sh], loT[:, :t_sh]
                    )
                for dch in range(2):
                    hiT = stp.tile([128, big], BF16, tag="hiT")
                    loT = stp.tile([128, big], BF16, tag="loT")
                    nc.sync.dma_start(
                        hiT[:, :s_len],
                        hs_hi_d.ap()[:, dch * 128 : (dch + 1) * 128],
                        transpose=True,
                    )
                    nc.sync.dma_start(
                        loT[:, :s_len],
                        hs_lo_d.ap()[:, dch * 128 : (dch + 1) * 128],
                        transpose=True,
                    )
                    nc.vector.tensor_add(
                        hsT_sb[:, dch, :], hiT[:, :s_len], loT[:, :s_len]
                    )

                # ---- G^T = (h_t @ W_in)^T : [d, q] ----
                for dt_ in range(2):
                    for q0 in range(0, t_sh, 512):
                        qn = min(512, t_sh - q0)
                        ps = sp.tile(
                            [128, q_blk if q_blk > 512 else 512], F32, tag="sps"
                        )
                        for tch in range(2):
                            nc.tensor.matmul(
                                ps[:, :qn],
                                w_sb[:, tch, dt_ * 128 : (dt_ + 1) * 128],
                                htT_sb[:, tch, q0 : q0 + qn],
                                start=(tch == 0),
                                stop=(tch == 1),
                            )
                        nc.vector.tensor_copy(gt_sb[:, dt_, q0 : q0 + qn], ps[:, :qn])

            # ---- main loop over q blocks ----
            for qb in range(nqb):
                q0 = qb * q_blk
                p_blk = bp.tile([128, nst * q_blk], BF16, tag="pblk")
                for st in range(nst):
                    ps = sp.tile([128, q_blk if q_blk > 512 else 512], F32, tag="sps")
                    for dch in range(2):
                        nc.tensor.matmul(
                            ps[:, :q_blk],
                            hsT_sb[:, dch, st * 128 : (st + 1) * 128],
                            gt_sb[:, dch, q0 : q0 + q_blk],
                            start=(dch == 0),
                            stop=(dch == 1),
                        )
                    nc.scalar.activation(
                        p_blk[:, st * q_blk : (st + 1) * q_blk],
                        ps[:, :q_blk],
                        EXP,
                        bias=mb_sb[:, st : st + 1],
                        scale=1.0,
                    )
                for qt in range(qsub):
                    qq = q0 + qt * 128
                    pv = vp.tile([128, d + 2], F32, tag="pv")
                    for st in range(nst):
                        nc.tensor.matmul(
                            pv[:],
                            p_blk[
                                :,
                                st * q_blk + qt * 128 : st * q_blk + (qt + 1) * 128,
                            ],
                            hsa16_sb[:, st, :],
                            start=(st == 0),
                            stop=(st == nst - 1),
                        )
                    invz = wp.tile([128, 1], F32, tag="invz")
                    nc.vector.reciprocal(invz[:], pv[:, d : d + 1])
                    cxt = wp.tile([128, d], F32, tag="cxt")
                    nc.vector.tensor_scalar_mul(cxt[:], pv[:, 0:d], invz[:])
                    nc.sync.dma_start(cx_d.ap()[qq : qq + 128, :], cxt[:])

                    pout = op.tile([128, s_len], BF16, tag="pout")
                    for sg in range(0, nst, 4):
                        n = min(4, nst - sg)
                        tp = tpp.tile([128, 512], BF16, tag="tps16")
                        for k in range(n):
                            st = sg + k
                            nc.tensor.transpose(
                                tp[:, k * 128 : (k + 1) * 128],
                                p_blk[
                                    :,
                                    st * q_blk
                                    + qt * 128 : st * q_blk
                                    + (qt + 1) * 128,
                                ],
                                id16_sb[:],
                            )
                        nc.vector.tensor_scalar_mul(
                            pout[:, sg * 128 : (sg + n) * 128],
                            tp[:, : n * 128],
                            invz[:],
                        )
                    half = s_len // 2
                    nc.gpsimd.dma_start(
                        sc_d.ap()[qq : qq + 128, :half], pout[:, :half]
                    )
                    nc.gpsimd.dma_start(
                        sc_d.ap()[qq : qq + 128, half:], pout[:, half:]
                    )

    if split_waits:
        _split_multi_waits(nc)
    return nc


_NC_CACHE = {}


def _get_nc():
    if "nc" not in _NC_CACHE:
        _NC_CACHE["nc"] = build_nc()
    return _NC_CACHE["nc"]


def _install_ntff_hook():
    """Dev-only: recreate the missing antenv.axon_hooks NTFF profile hook via
    ctypes on the injected axon .so, so trace=True yields exec_time_ns."""
    import contextlib
    import ctypes
    import sys
    import types

    try:
        import antenv.axon_hooks  # noqa: F401

        return
    except ImportError:
        pass

    so_path = "/opt/axon/libaxon_pjrt.so"
    hook = None
    try:
        lib = ctypes.CDLL(so_path)
        if hasattr(lib, "axon_start_nrt_profile"):
            lib.axon_start_nrt_profile.argtypes = [
                ctypes.POINTER(ctypes.c_int64),
                ctypes.c_size_t,
            ]
            lib.axon_start_nrt_profile.restype = ctypes.c_int64
            lib.axon_stop_nrt_profile.argtypes = [ctypes.c_char_p]
            lib.axon_stop_nrt_profile.restype = ctypes.c_int64

            @contextlib.contextmanager
            def _hook(output_dir, device_ids):
                import jax

                jax.devices()
                if device_ids:
                    ids = (ctypes.c_int64 * len(device_ids))(*device_ids)
                    rc = lib.axon_start_nrt_profile(ids, len(device_ids))
                else:
                    rc = lib.axon_start_nrt_profile(None, 0)
                if rc != 0:
                    raise RuntimeError(f"axon_start_nrt_profile rc={rc}")
                try:
                    yield
                finally:
                    n = lib.axon_stop_nrt_profile(str(output_dir).encode())
                    if n < 0:
                        raise RuntimeError(f"axon_stop_nrt_profile rc={n}")
                    print(f"profile: {n} file(s) written to {output_dir}", file=sys.stderr)

            hook = _hook
    except OSError:
        pass

    mod = types.ModuleType("antenv.axon_hooks")
    mod._hook = hook
    mod.get_axon_ntff_profile_hook = lambda: mod._hook
    mod.set_axon_ntff_profile_hook = lambda h: setattr(mod, "_hook", h)
    sys.modules["antenv.axon_hooks"] = mod
    import antenv

    antenv.axon_hooks = mod

    # artifact upload has no destination in this sandbox
    import concourse.bass_utils as bu

    bu.upload_artifacts = lambda tmpdir: "local://skipped"


def kernel(h_t, h_s, m_s, W_in):
    h_t = np.ascontiguousarray(np.asarray(h_t, dtype=np.float32))
    h_s = np.ascontiguousarray(np.asarray(h_s, dtype=np.float32))
    m_s = np.asarray(m_s)
    W_in = np.ascontiguousarray(np.asarray(W_in, dtype=np.float32))

    nc = _get_nc()
    import ml_dtypes

    ident16 = np.eye(128, dtype=ml_dtypes.bfloat16)

    def hilo(x):
        hi = x.astype(ml_dtypes.bfloat16)
        lo = (x - hi.astype(np.float32)).astype(ml_dtypes.bfloat16)
        return np.ascontiguousarray(hi), np.ascontiguousarray(lo)
    aug = np.zeros((SRC, 2), dtype=np.float32)
    aug[:, 0] = 1.0
    hsa = [np.ascontiguousarray(np.concatenate([h_s[b], aug], axis=1)) for b in range(B)]
    hsa16 = [np.ascontiguousarray(a.astype(ml_dtypes.bfloat16)) for a in hsa]
    hs_hilo = [hilo(h_s[b]) for b in range(B)]
    mb = [
        np.ascontiguousarray((m_s[b].astype(np.float32) - 1.0) * NEG_BIG)
        for b in range(B)
    ]

    in_maps = []
    for c in range(N_CORES):
        b, half = c // 2, c % 2
        ht_hi, ht_lo = hilo(h_t[b, half * T_SH : (half + 1) * T_SH, :])
        in_maps.append(
            {
                "maskbias": mb[b],
                "W_in": W_in,
                "ident16": ident16,
                "h_s_aug16": hsa16[b],
                "hs_hi": hs_hilo[b][0],
                "hs_lo": hs_hilo[b][1],
                "ht_hi": ht_hi,
                "ht_lo": ht_lo,
            }
        )

    trace = os.environ.get("TRNK_TRACE", "0") == "1"
    if trace:
        _install_ntff_hook()
    res = run_bass_kernel_spmd(
        nc, in_maps, core_ids=list(range(N_CORES)), trace=trace
    )
    kernel.last_exec_time_ns = res.exec_time_ns
    kernel.last_results = res

    scores = np.empty((B, TGT, SRC), dtype=np.float32)
    context = np.empty((B, TGT, D), dtype=np.float32)
    for c in range(N_CORES):
        b, half = c // 2, c % 2
        scores[b, half * T_SH : (half + 1) * T_SH, :] = res.results[c]["scores"]
        context[b, half * T_SH : (half + 1) * T_SH, :] = res.results[c]["context"]
    return context, scores


# revision 18
# speedup vs baseline: 1.5053x; 1.0293x over previous
"""Trainium2 Bass kernel for masked Luong-'general' attention.

reference math (per batch b):
    scores = softmax(mask(h_t @ W_in @ h_s^T))       # [T, S]
    context = scores @ h_s                           # [T, D]
returns (context, scores).

Sharding: 8 cores = 4 batches x 2 halves of the target sequence.
Each core computes a [2048, 4096] score block + [2048, 256] context block.

Per-core dataflow (all matmuls in float32r = fp22, full PE rate):
  G^T[d,q]   = W_in(native lhsT) . h_tT          (h_tT via PE transposes)
  S_T[s,q]   = h_sT-chunks(lhsT) . G^T           (PSUM)
  P_T[s,q]   = exp(S_T + maskbias[s])            (one ACT op: mask+exp+drain)
  ctx|Z[q,.] = P_T-chunks(lhsT) . [h_s | 1]      (ones col gives softmax denom)
  scores     = transpose(P_T) * (1/Z)            (PE transpose + DVE drain)
"""

import os
import numpy as np

import concourse.bass as bass
import concourse.tile as tile
import concourse.mybir as mybir
from concourse.bass_utils import run_bass_kernel_spmd

F32 = mybir.dt.float32
F32R = mybir.dt.float32r
BF16 = mybir.dt.bfloat16
EXP = mybir.ActivationFunctionType.Exp

N_CORES = 8
B, TGT, SRC, D = 4, 4096, 4096, 256
T_SH = TGT * B // N_CORES  # 2048 target rows per core
NEG_BIG = 1.0e11


def _r(ap):
    return ap.bitcast(F32R)


def _split_multi_waits(nc, max_waits=1):
    """This walrus build's CTRL templates (Drain etc.) only accept one sync
    wait; peel extra waits onto preceding NoOps on the same engine."""
    for f in nc.m.functions:
        for bb in f.blocks:
            insts = bb.instructions
            i = 0
            while i < len(insts):
                inst = insts[i]
                si = inst.sync_info
                if si is not None and len(si.on_wait) > max_waits:
                    waits = list(si.on_wait)
                    keep = waits[-max_waits:]
                    extra = waits[:-max_waits]
                    nops = []
                    for j in range(0, len(extra), max_waits):
                        nops.append(
                            mybir.InstNoOp(
                                name=f"{inst.name}_ws{j}",
                                engine=inst.engine,
                                bass_nofuse=True,
                                sync_info=mybir.SyncInfo(
                                    on_wait=extra[j : j + max_waits], on_update=[]
                                ),
                            )
                        )
                    inst.sync_info = mybir.SyncInfo(
                        on_wait=keep, on_update=list(si.on_update)
                    )
                    for k, nop in enumerate(nops):
                        insts.insert(i + k, nop)
                    i += len(nops)
                i += 1


def build_nc(t_sh=T_SH, s_len=SRC, q_blk=512, split_waits=True):
    """Build the per-core SPMD program. All cores run the same NEFF."""
    d = D
    nst = s_len // 128  # s tiles
    nqb = t_sh // q_blk  # q blocks held in SBUF at once
    qsub = q_blk // 128  # q subtiles per block

    nc = bass.Bass(trn_type="TRN2", target_bir_lowering=False, debug=False)

    mb_d = nc.dram_tensor("maskbias", [s_len], F32, kind="ExternalInput")
    w_d = nc.dram_tensor("W_in", [d, d], F32R, kind="ExternalInput")
    id16_d = nc.dram_tensor("ident16", [128, 128], BF16, kind="ExternalInput")
    hsa16_d = nc.dram_tensor("h_s_aug16", [s_len, d + 2], BF16, kind="ExternalInput")
    hs_hi_d = nc.dram_tensor("hs_hi", [2, s_len, 128], BF16, kind="ExternalInput")
    hs_lo_d = nc.dram_tensor("hs_lo", [2, s_len, 128], BF16, kind="ExternalInput")
    ht_hi_d = nc.dram_tensor("ht_hi", [2, t_sh, 128], BF16, kind="ExternalInput")
    ht_lo_d = nc.dram_tensor("ht_lo", [2, t_sh, 128], BF16, kind="ExternalInput")
    sc_d = nc.dram_tensor("scores", [t_sh, s_len], F32, kind="ExternalOutput")
    cx_d = nc.dram_tensor("context", [t_sh, d], F32, kind="ExternalOutput")

    with tile.TileContext(nc) as tc:
        with (
            tc.tile_pool(name="persist", bufs=1) as pp,
            tc.tile_pool(name="pblk", bufs=2) as bp,
            tc.tile_pool(name="pout", bufs=2) as op,
            tc.tile_pool(name="work", bufs=2) as wp,
            tc.tile_pool(name="spsum", bufs=3, space="PSUM") as sp,
            tc.tile_pool(name="tpsum", bufs=2, space="PSUM") as tpp,
            tc.tile_pool(name="vpsum", bufs=2, space="PSUM") as vp,
        ):
            # ---- persistent tiles (v4 allocation order) ----
            w_sb = pp.tile([128, 2, d], F32R, tag="w")
            nc.sync.dma_start(w_sb[:], w_d.ap().rearrange("(c p) f -> p c f", p=128))
            id16_sb = pp.tile([128, 128], BF16, tag="ident16")
            nc.sync.dma_start(id16_sb[:], id16_d.ap())
            hsa16_sb = pp.tile([128, nst, d + 2], BF16, tag="hsa16")
            mb_sb = pp.tile([128, nst], F32, tag="mb")
            nc.sync.dma_start(mb_sb[:], mb_d.ap().rearrange("(c p) -> p c", p=128))
            hsT_sb = pp.tile([128, 2, s_len], F32R, tag="hsT")
            gt_sb = pp.tile([128, 2, t_sh], F32R, tag="gt")

            # ---- transposed input loads (xbar DMA, hi/lo bf16 split) ----
            with tc.tile_pool(name="setup", bufs=1) as stp:
                htT_sb = stp.tile([128, 2, t_sh], F32R, tag="htT")
                big = s_len if s_len >= t_sh else t_sh
                for dch in range(2):
                    hiT = stp.tile([128, big], BF16, tag="hiT")
                    loT = stp.tile([128, big], BF16, tag="loT")
                    nc.sync.dma_start(
                        hiT[:, :t_sh],
                        ht_hi_d.ap()[dch],
                        transpose=True,
                    )
                    nc.sync.dma_start(
                        loT[:, :t_sh],
                        ht_lo_d.ap()[dch],
                        transpose=True,
                    )
                    nc.vector.tensor_add(
                        htT_sb[:, dch, :], hiT[:, :t_sh], loT[:, :t_sh]
                    )
                for dch in range(2):
                    hiT = stp.tile([128, big], BF16, tag="hiT")
                    loT = stp.tile([128, big], BF16, tag="loT")
                    nc.sync.dma_start(
                        hiT[:, :s_len],
                        hs_hi_d.ap()[dch],
                        transpose=True,
                    )
                    nc.sync.dma_start(
                        loT[:, :s_len],
                        hs_lo_d.ap()[dch],
                        transpose=True,
                    )
                    nc.vector.tensor_add(
                        hsT_sb[:, dch, :], hiT[:, :s_len], loT[:, :s_len]
                    )

                nc.gpsimd.dma_start(
                    hsa16_sb[:], hsa16_d.ap().rearrange("(c p) f -> p c f", p=128)
                )

                # ---- G^T = (h_t @ W_in)^T : [d, q] ----
                for dt_ in range(2):
                    for q0 in range(0, t_sh, 512):
                        qn = min(512, t_sh - q0)
                        ps = sp.tile(
                            [128, q_blk if q_blk > 512 else 512], F32, tag="sps"
                        )
                        for tch in range(2):
                            nc.tensor.matmul(
                                ps[:, :qn],
                                w_sb[:, tch, dt_ * 128 : (dt_ + 1) * 128],
                                htT_sb[:, tch, q0 : q0 + qn],
                                start=(tch == 0),
                                stop=(tch == 1),
                            )
                        nc.vector.tensor_copy(gt_sb[:, dt_, q0 : q0 + qn], ps[:, :qn])

            # ---- main loop over q blocks ----
            for qb in range(nqb):
                q0 = qb * q_blk
                p_blk = bp.tile([128, nst * q_blk], BF16, tag="pblk")
                for st in range(nst):
                    ps = sp.tile([128, q_blk if q_blk > 512 else 512], F32, tag="sps")
                    for dch in range(2):
                        nc.tensor.matmul(
                            ps[:, :q_blk],
                            hsT_sb[:, dch, st * 128 : (st + 1) * 128],
                            gt_sb[:, dch, q0 : q0 + q_blk],
                            start=(dch == 0),
                            stop=(dch == 1),
                        )
                    nc.scalar.activation(
                        p_blk[:, st * q_blk : (st + 1) * q_blk],
                        ps[:, :q_blk],
                        EXP,
                        bias=mb_sb[:, st : st + 1],
                        scale=1.0,
                    )
                for qt in range(qsub):
                    qq = q0 + qt * 128
                    pv = vp.tile([128, d + 2], F32, tag="pv")
                    for st in range(nst):
                        nc.tensor.matmul(
                            pv[:],
                            p_blk[
                                :,
                                st * q_blk + qt * 128 : st * q_blk + (qt + 1) * 128,
                            ],
                            hsa16_sb[:, st, :],
                            start=(st == 0),
                            stop=(st == nst - 1),
                        )
                    invz = wp.tile([128, 1], F32, tag="invz")
                    nc.vector.reciprocal(invz[:], pv[:, d : d + 1])
                    cxt = wp.tile([128, d], F32, tag="cxt")
                    nc.vector.tensor_scalar_mul(cxt[:], pv[:, 0:d], invz[:])
                    nc.sync.dma_start(cx_d.ap()[qq : qq + 128, :], cxt[:])

                    pout = op.tile([128, s_len], BF16, tag="pout")
                    for sg in range(0, nst, 4):
                        n = min(4, nst - sg)
                        tp = tpp.tile([128, 512], BF16, tag="tps16")
                        for k in range(n):
                            st = sg + k
                            nc.tensor.transpose(
                                tp[:, k * 128 : (k + 1) * 128],
                                p_blk[
                                    :,
                                    st * q_blk
                                    + qt * 128 : st * q_blk
                                    + (qt + 1) * 128,
                                ],
                                id16_sb[:],
                            )
                        nc.vector.tensor_scalar_mul(
                            pout[:, sg * 128 : (sg + n) * 128],
                            tp[:, : n * 128],
                            invz[:],
                        )
                    qtr = max(s_len // 4, 128)
                    for o in range(0, s_len, qtr):
                        nc.gpsimd.dma_start(
                            sc_d.ap()[qq : qq + 128, o : o + qtr],
                            pout[:, o : o + qtr],
                        )

    if split_waits:
        _split_multi_waits(nc)
    return nc


_NC_CACHE = {}


def _get_nc():
    if "nc" not in _NC_CACHE:
        _NC_CACHE["nc"] = build_nc()
    return _NC_CACHE["nc"]


def _install_ntff_hook():
    """Dev-only: recreate the missing antenv.axon_hooks NTFF profile hook via
    ctypes on the injected axon .so, so trace=True yields exec_time_ns."""
    import contextlib
    import ctypes
    import sys
    import types

    try:
        import antenv.axon_hooks  # noqa: F401

        return
    except ImportError:
        pass

    so_path = "/opt/axon/libaxon_pjrt.so"
    hook = None
    try:
        lib = ctypes.CDLL(so_path)
        if hasattr(lib, "axon_start_nrt_profile"):
            lib.axon_start_nrt_profile.argtypes = [
                ctypes.POINTER(ctypes.c_int64),
                ctypes.c_size_t,
            ]
            lib.axon_start_nrt_profile.restype = ctypes.c_int64
            lib.axon_stop_nrt_profile.argtypes = [ctypes.c_char_p]
            lib.axon_stop_nrt_profile.restype = ctypes.c_int64

            @contextlib.contextmanager
            def _hook(output_dir, device_ids):
                import jax

                jax.devices()
                if device_ids:
                    ids = (ctypes.c_int64 * len(device_ids))(*device_ids)
                    rc = lib.axon_start_nrt_profile(ids, len(device_ids))
                else:
                    rc = lib.axon_start_nrt_profile(None, 0)
                if rc != 0:
                    raise RuntimeError(f"axon_start_nrt_profile rc={rc}")
                try:
                    yield
                finally:
                    n = lib.axon_stop_nrt_profile(str(output_dir).encode())
                    if n < 0:
                        raise RuntimeError(f"axon_stop_nrt_profile rc={n}")
                    print(f"profile: {n} file(s) written to {output_dir}", file=sys.stderr)

            hook = _hook
    except OSError:
        pass

    mod = types.ModuleType("antenv.axon_hooks")
    mod._hook = hook
    mod.get_axon_ntff_profile_hook = lambda: mod._hook
    mod.set_axon_ntff_profile_hook = lambda h: setattr(mod, "_hook", h)
    sys.modules["antenv.axon_hooks"] = mod
    import antenv

    antenv.axon_hooks = mod

    # artifact upload has no destination in this sandbox
    import concourse.bass_utils as bu

    bu.upload_artifacts = lambda tmpdir: "local://skipped"


def kernel(h_t, h_s, m_s, W_in):
    h_t = np.ascontiguousarray(np.asarray(h_t, dtype=np.float32))
    h_s = np.ascontiguousarray(np.asarray(h_s, dtype=np.float32))
    m_s = np.asarray(m_s)
    W_in = np.ascontiguousarray(np.asarray(W_in, dtype=np.float32))

    nc = _get_nc()
    import ml_dtypes

    ident16 = np.eye(128, dtype=ml_dtypes.bfloat16)

    def hilo(x):
        hi = x.astype(ml_dtypes.bfloat16)
        lo = (x - hi.astype(np.float32)).astype(ml_dtypes.bfloat16)
        cm = lambda a: np.ascontiguousarray(
            a.reshape(-1, 2, 128).transpose(1, 0, 2)
        )
        return cm(hi), cm(lo)
    aug = np.zeros((SRC, 2), dtype=np.float32)
    aug[:, 0] = 1.0
    hsa = [np.ascontiguousarray(np.concatenate([h_s[b], aug], axis=1)) for b in range(B)]
    hsa16 = [np.ascontiguousarray(a.astype(ml_dtypes.bfloat16)) for a in hsa]
    hs_hilo = [hilo(h_s[b]) for b in range(B)]
    mb = [
        np.ascontiguousarray((m_s[b].astype(np.float32) - 1.0) * NEG_BIG)
        for b in range(B)
    ]

    in_maps = []
    for c in range(N_CORES):
        b, half = c // 2, c % 2
        ht_hi, ht_lo = hilo(h_t[b, half * T_SH : (half + 1) * T_SH, :])
        in_maps.append(
            {
                "maskbias": mb[b],
                "W_in": W_in,
                "ident16": ident16,
                "h_s_aug16": hsa16[b],
                "hs_hi": hs_hilo[b][0],
                "hs_lo": hs_hilo[b][1],
                "ht_hi": ht_hi,
                "ht_lo": ht_lo,
            }
        )

    trace = os.environ.get("TRNK_TRACE", "0") == "1"
    if trace:
        _install_ntff_hook()
    res = run_bass_kernel_spmd(
        nc, in_maps, core_ids=list(range(N_CORES)), trace=trace
    )
    kernel.last_exec_time_ns = res.exec_time_ns
    kernel.last_results = res

    scores = np.empty((B, TGT, SRC), dtype=np.float32)
    context = np.empty((B, TGT, D), dtype=np.float32)
    for c in range(N_CORES):
        b, half = c // 2, c % 2
        scores[b, half * T_SH : (half + 1) * T_SH, :] = res.results[c]["scores"]
        context[b, half * T_SH : (half + 1) * T_SH, :] = res.results[c]["context"]
    return context, scores
